# revision 1
# baseline (speedup 1.0000x reference)
"""Trainium2 Bass kernel for BlockSparseMoE (DeepSeek-V2-Lite-like MoE layer).

Strategy (8 NeuronCores, SPMD):
  * Router (softmax + grouped top-k over [2048, 64] scores) is computed on
    host in numpy - it is ~0.03% of the FLOPs; selection matches the jax
    reference exactly on the fixed inputs.
  * Routed experts: expert-parallel. Each core owns 8 of the 64 experts.
    Tokens are gathered per expert on host (experts rank-matched across
    cores so that slot j has the same token capacity on every core, keeping
    the SPMD program shape-uniform while minimizing padding), shipped
    pre-transposed; each core runs gate/up GEMM -> silu*mul -> down GEMM ->
    per-token combine-weight scaling for its experts.
  * Shared experts: 2D-sharded (intermediate-dim quarter x token half per
    core); partial outputs are summed on host together with the gathered
    routed outputs (the host-side unshard).
"""

import os
import sys
import math

sys.path.insert(0, "/opt/trn_rl_repo")

import numpy as np
import ml_dtypes

import concourse.bass as bass  # noqa: F401  (registers AP machinery)
import concourse.mybir as mybir
import concourse.tile as tile
from concourse import bacc
from concourse import bass_utils

# Model dims (hardcoded per spec)
M = 2048
H = 1024
E = 64
N = 1024
N_GROUP = 8
TOPK_GROUP = 3
TOPK = 6
IS = 2048          # shared-expert intermediate size (n_shared=2 * N)
NCORES = 8
ISS = IS // 4       # per-core shared-expert intermediate slice (2D shard)
MH = M // 2         # per-core shared-expert token half
MAXC = 512          # max tokens per expert slot (fp32 moving-dim limit)

# Compute dtype for the GEMMs: "bf16" (fast) or "f32r" (tf32-like, ~16x more
# accurate, ~2x HBM traffic).
COMPUTE_DTYPE = os.environ.get("KERNEL_DTYPE", "bf16")
# Output dtype for yw/sh partials ("bf16" halves output DMA bytes; rows are
# combine-weighted so bf16's scale-free rounding costs ~1e-3 relative).
OUT_DTYPE = os.environ.get("KERNEL_OUT_DTYPE", "bf16")

P = 128
KT = H // P    # 8 k-tiles for H contraction
NT = N // P    # 8 n-tiles for N contraction
OC = H // 512  # 2 output column chunks
FT = ISS // P  # shared-expert f-tiles per gate/up half (4)


# ---------------------------------------------------------------- routing ---
def _route(x, gate_w):
    """Numpy replica of the reference router. Returns topk ids/weights."""
    logits = x @ gate_w.T                          # [M, E] fp32 sgemm
    mx = logits.max(-1, keepdims=True)
    ex = np.exp(logits - mx)
    scores = ex / ex.sum(-1, keepdims=True)        # softmax, [M, E]
    m = scores.shape[0]
    gs = scores.reshape(m, N_GROUP, E // N_GROUP).max(-1)        # [M, G]
    gidx = np.argsort(-gs, axis=-1, kind="stable")[:, :TOPK_GROUP]
    gmask = np.zeros((m, N_GROUP), bool)
    gmask[np.arange(m)[:, None], gidx] = True
    smask = np.repeat(gmask, E // N_GROUP, axis=1)               # [M, E]
    tmp = np.where(smask, scores, 0.0)
    topk_ids = np.argsort(-tmp, axis=-1, kind="stable")[:, :TOPK]
    topk_w = np.take_along_axis(tmp, topk_ids, axis=-1)
    return topk_w.astype(np.float32), topk_ids


# ------------------------------------------------------------ bass program ---
_prog_cache = {}


def _build_program(S, caps, dt_name):
    """One SPMD program: S expert slots (slot j holds caps[j] tokens) plus a
    shared-expert slice, with shared-expert chunks interleaved between slots
    so output DMAs overlap routed compute."""
    key = (S, tuple(caps), dt_name, OUT_DTYPE)
    if key in _prog_cache:
        return _prog_cache[key]

    DT = mybir.dt.bfloat16 if dt_name == "bf16" else mybir.dt.float32
    f32 = mybir.dt.float32
    CAPMAX = P * math.ceil(max(caps) / P)
    CTMAX = CAPMAX // P

    nc = bacc.Bacc("TRN2", target_bir_lowering=False, debug=False,
                   num_devices=NCORES)

    xg_offs = [0]
    for c in caps:
        xg_offs.append(xg_offs[-1] + P * KT * c)
    xgT = nc.dram_tensor("xgT", [xg_offs[-1]], DT, kind="ExternalInput")
    w1T = nc.dram_tensor("w1T", [S, H, 2 * N], DT, kind="ExternalInput")
    w2T = nc.dram_tensor("w2T", [S, N, H], DT, kind="ExternalInput")
    wts = nc.dram_tensor("wts", [S, CAPMAX], f32, kind="ExternalInput")
    xT = nc.dram_tensor("xT", [H, MH], DT, kind="ExternalInput")
    sguT = nc.dram_tensor("sguT", [H, 2 * ISS], DT, kind="ExternalInput")
    sdT = nc.dram_tensor("sdT", [ISS, H], DT, kind="ExternalInput")
    ODT = mybir.dt.bfloat16 if OUT_DTYPE == "bf16" else f32
    yw = nc.dram_tensor("yw", [S, CAPMAX, H], ODT, kind="ExternalOutput")
    sh = nc.dram_tensor("sh", [MH, H], ODT, kind="ExternalOutput")

    def mm_dt(ap):
        # reinterpret fp32 operands as float32r at the matmul for full PE rate
        return ap.bitcast(mybir.dt.float32r) if dt_name == "f32r" else ap

    with tile.TileContext(nc) as tc:
        with (
            tc.tile_pool(name="w1pool", bufs=2) as w1pool,
            tc.tile_pool(name="wpool", bufs=2) as wpool,
            tc.tile_pool(name="xpool", bufs=3) as xpool,
            tc.tile_pool(name="hpool", bufs=2) as hpool,
            tc.tile_pool(name="spool", bufs=1) as spool,
            tc.tile_pool(name="silu", bufs=3) as silu_pool,
            tc.tile_pool(name="opool",
                         bufs=2 if OUT_DTYPE == "bf16" else 1) as opool,
            tc.tile_pool(name="ps1", bufs=5, space="PSUM") as ps1,
            tc.tile_pool(name="ps2", bufs=3, space="PSUM") as ps2,
        ):
            state = {}

            def load_shared_inputs():
                sgu_sb = spool.tile([P, KT, 2 * ISS], DT, tag="sgu")
                sd_sb = spool.tile([P, FT, H], DT, tag="sd")
                state.update(sgu_sb=sgu_sb, sd_sb=sd_sb)

            def routed_slot(s):
                cap = caps[s]
                ct_n = math.ceil(cap / P)
                xg_sb = xpool.tile([P, KT, cap], DT, tag="xg", name="xg_sb")
                w1_sb = w1pool.tile([P, KT, 2 * N], DT, tag="w1")
                xg_src = xgT.ap()[xg_offs[s]:xg_offs[s + 1]].rearrange(
                    "(p kt c) -> p kt c", p=P, kt=KT)
                if s == 0:
                    for kt in range(KT):
                        nc.sync.dma_start(xg_sb[:, kt], xg_src[:, kt])
                        nc.sync.dma_start(
                            w1_sb[:, kt],
                            w1T.ap()[s].rearrange(
                                "(kt p) f -> p kt f", p=P)[:, kt])
                else:
                    nc.sync.dma_start(xg_sb[:], xg_src)
                    for half in range(2):
                        hs = slice(half * (KT // 2), (half + 1) * (KT // 2))
                        nc.sync.dma_start(
                            w1_sb[:, hs],
                            w1T.ap()[s].rearrange(
                                "(kt p) f -> p kt f", p=P)[:, hs])
                w2_sb = wpool.tile([P, NT, H], DT, tag="w2")
                nc.sync.dma_start(
                    w2_sb[:], w2T.ap()[s].rearrange("(nt p) o -> p nt o", p=P))
                wt_sb = xpool.tile([P, CTMAX], f32, tag="wt")
                nc.sync.dma_start(
                    wt_sb[:], wts.ap()[s].rearrange("(ct p) -> p ct", p=P))

                # gate/up GEMM -> silu*mul, producing hT [n, c] in SBUF
                h_sb = hpool.tile([P, NT, CAPMAX], DT, tag="h")
                for nt in range(NT):
                    pg = ps1.tile([P, 512], f32, tag="ps1", name="pg")[:, :cap]
                    pu = ps1.tile([P, 512], f32, tag="ps1", name="pu")[:, :cap]
                    for kt in range(KT):
                        nc.tensor.matmul(
                            pg, mm_dt(w1_sb[:, kt, nt * P:(nt + 1) * P]),
                            mm_dt(xg_sb[:, kt]),
                            start=(kt == 0), stop=(kt == KT - 1))
                    for kt in range(KT):
                        nc.tensor.matmul(
                            pu,
                            mm_dt(w1_sb[:, kt, N + nt * P:N + (nt + 1) * P]),
                            mm_dt(xg_sb[:, kt]),
                            start=(kt == 0), stop=(kt == KT - 1))
                    st = silu_pool.tile([P, 512], f32, tag="silu", name="st")[:, :cap]
                    nc.scalar.activation(
                        st, pg, mybir.ActivationFunctionType.Silu)
                    nc.vector.tensor_mul(h_sb[:, nt, :cap], st, pu)

                # down GEMM + combine-weight scaling -> yw
                ow = opool.tile([P, CTMAX, H], ODT, tag="owb", name="ow")
                for ct in range(ct_n):
                    pt = min(P, cap - ct * P)
                    for oc in range(OC):
                        po = ps2.tile([P, 512], f32, tag="ps2", name="po")[:pt]
                        for nt in range(NT):
                            nc.tensor.matmul(
                                po,
                                mm_dt(h_sb[:, nt, ct * P:ct * P + pt]),
                                mm_dt(w2_sb[:, nt, oc * 512:(oc + 1) * 512]),
                                start=(nt == 0), stop=(nt == NT - 1))
                        nc.scalar.activation(
                            ow[:pt, ct, oc * 512:(oc + 1) * 512], po,
                            mybir.ActivationFunctionType.Copy,
                            scale=wt_sb[:pt, ct:ct + 1])
                ptl = cap - (ct_n - 1) * P  # rows in the last c-tile
                if s == S - 1:
                    for ct in range(ct_n):
                        pt = min(P, cap - ct * P)
                        for oc in range(OC):
                            nc.scalar.dma_start(
                                yw.ap()[s, ct * P:ct * P + pt,
                                        oc * 512:(oc + 1) * 512],
                                ow[:pt, ct, oc * 512:(oc + 1) * 512])
                else:
                    if ct_n > 1:
                        nc.scalar.dma_start(
                            yw.ap()[s].rearrange(
                                "(ct p) o -> p ct o", p=P)[:, :ct_n - 1],
                            ow[:, :ct_n - 1])
                    nc.scalar.dma_start(
                        yw.ap()[s, (ct_n - 1) * P:(ct_n - 1) * P + ptl],
                        ow[:ptl, ct_n - 1])

            sh_state = {}

            def shared_g1(off, W):
                xT_sb = xpool.tile([P, KT, 512], DT, tag="xTc",
                                   name="xT_sb")[:, :, :W]
                xT_r = xT.ap().rearrange("(kt p) m -> p kt m", p=P)
                if off == 0:
                    load_shared_inputs()
                    sgu_sb = state["sgu_sb"]
                    sgu_r = sguT.ap().rearrange("(kt p) f -> p kt f", p=P)
                    # interleave per-kt so the first matmuls start early
                    for kt in range(KT):
                        nc.sync.dma_start(xT_sb[:, kt],
                                          xT_r[:, kt, off:off + W])
                        nc.sync.dma_start(sgu_sb[:, kt], sgu_r[:, kt])
                else:
                    sgu_sb = state["sgu_sb"]
                    nc.sync.dma_start(
                        xT_sb[:], xT_r[:, :, off:off + W])
                sh_h = hpool.tile([P, FT, 512], DT, tag="shh",
                                  name="sh_h")[:, :, :W]
                sh_state[off] = sh_h
                for i in range(FT):
                    pg = ps1.tile([P, 512], f32, tag="ps1", name="pgs")[:, :W]
                    pu = ps1.tile([P, 512], f32, tag="ps1", name="pus")[:, :W]
                    for kt in range(KT):
                        nc.tensor.matmul(
                            pg, mm_dt(sgu_sb[:, kt, i * P:(i + 1) * P]),
                            mm_dt(xT_sb[:, kt]),
                            start=(kt == 0), stop=(kt == KT - 1))
                    for kt in range(KT):
                        nc.tensor.matmul(
                            pu,
                            mm_dt(sgu_sb[:, kt, ISS + i * P:ISS + (i + 1) * P]),
                            mm_dt(xT_sb[:, kt]),
                            start=(kt == 0), stop=(kt == KT - 1))
                    st = silu_pool.tile([P, 512], f32, tag="silu",
                                        name="sts")[:, :W]
                    nc.scalar.activation(
                        st, pg, mybir.ActivationFunctionType.Silu)
                    nc.vector.tensor_mul(sh_h[:, i], st, pu)

            def shared_g2(off, W):
                if "sd_loaded" not in sh_state:
                    sh_state["sd_loaded"] = True
                    nc.sync.dma_start(
                        state["sd_sb"][:],
                        sdT.ap().rearrange("(nt p) o -> p nt o", p=P))
                sd_sb = state["sd_sb"]
                sh_h = sh_state.pop(off)
                os_ = opool.tile([P, 4, H], ODT, tag="oshb",
                                 name="os_")[:, :W // P]
                for ctc in range(W // P):
                    for oc in range(OC):
                        po = ps2.tile([P, 512], f32, tag="ps2")
                        for nt2 in range(FT):
                            nc.tensor.matmul(
                                po[:],
                                mm_dt(sh_h[:, nt2, ctc * P:(ctc + 1) * P]),
                                mm_dt(sd_sb[:, nt2, oc * 512:(oc + 1) * 512]),
                                start=(nt2 == 0), stop=(nt2 == FT - 1))
                        nc.vector.tensor_copy(
                            os_[:, ctc, oc * 512:(oc + 1) * 512], po[:])
                    nc.scalar.dma_start(
                        sh.ap()[off + ctc * P:off + (ctc + 1) * P],
                        os_[:, ctc])


            # schedule: a shared chunk first (small input footprint covers
            # the weight-stream ramp), one mid-stream as DMA relief, and end
            # on a routed slot to keep the drain tail short.
            if S >= 5:
                sched = [("g1", (0, 512)), ("slot", 0), ("g2", (0, 512)),
                         ("g1", (512, 512)), ("slot", 1), ("slot", 2),
                         ("g2", (512, 512))]
                sched += [("slot", s) for s in range(3, S)]
            else:
                sched = [("g1", (0, 512)), ("g2", (0, 512)),
                         ("g1", (512, 512)), ("g2", (512, 512))]
                sched += [("slot", s) for s in range(S)]
            for kind, idx in sched:
                if kind == "slot":
                    routed_slot(idx)
                elif kind == "g1":
                    shared_g1(*idx)
                else:
                    shared_g2(*idx)

    nc.compile()
    _prog_cache[key] = nc
    return nc


# ------------------------------------------------------------------ kernel ---
def _prepare(x, gate_w, w1, w2, shared_gate_up, shared_down):
    x = np.ascontiguousarray(np.asarray(x, np.float32))
    gate_w = np.asarray(gate_w, np.float32)
    w1 = np.asarray(w1, np.float32)
    w2 = np.asarray(w2, np.float32)
    shared_gate_up = np.asarray(shared_gate_up, np.float32)
    shared_down = np.asarray(shared_down, np.float32)

    dt_name = COMPUTE_DTYPE
    np_dt = ml_dtypes.bfloat16 if dt_name == "bf16" else np.float32

    # ---- host router + dispatch build
    topk_w, topk_ids = _route(x, gate_w)
    order = np.argsort(topk_ids, axis=None, kind="stable")  # stable (t, k) order
    flat_ids = topk_ids.ravel()[order]
    flat_tok = (np.arange(M * TOPK) // TOPK)[order]
    flat_w = topk_w.ravel()[order]
    starts = np.searchsorted(flat_ids, np.arange(E + 1))
    chunks = []  # (ntok, expert, tokens, weights)
    for e in range(E):
        t = flat_tok[starts[e]:starts[e + 1]]
        w = flat_w[starts[e]:starts[e + 1]]
        for i in range(0, max(len(t), 1), MAXC):
            chunks.append((len(t[i:i + MAXC]), e, t[i:i + MAXC], w[i:i + MAXC]))

    # rank-match chunks across cores: sort by size, chunk ranked r goes to
    # core r%8, slot r//8 -> slot j has capacity max(sizes of ranks 8j..8j+7)
    chunks.sort(key=lambda c: -c[0])
    S = math.ceil(len(chunks) / NCORES)
    while len(chunks) < S * NCORES:
        chunks.append((0, 0, np.zeros(0, np.int64), np.zeros(0, np.float32)))
    caps = [max(4, chunks[j * NCORES][0]) for j in range(S)]
    if dt_name == "f32r":
        caps = [max(c, 256) for c in caps]  # float32r full-rate needs >=256

    nc = _build_program(S, caps, dt_name)
    CAPMAX = P * math.ceil(max(caps) / P)

    # ---- per-core input maps
    xT_np = np.ascontiguousarray(x.T).astype(np_dt)
    in_maps = []
    inv = np.zeros((M, TOPK), np.int64)
    cnt = np.zeros(M, np.int32)
    KTc = H // P
    xg_offs = [0]
    for c in caps:
        xg_offs.append(xg_offs[-1] + P * KTc * c)
    for core in range(NCORES):
        xgT = np.zeros(xg_offs[-1], np_dt)
        w1T = np.zeros((S, H, 2 * N), np_dt)
        w2T = np.zeros((S, N, H), np_dt)
        wv = np.zeros((S, CAPMAX), np.float32)
        for j in range(S):
            _, e, t, w = chunks[j * NCORES + core]
            w1T[j] = w1[e].T.astype(np_dt)
            w2T[j] = w2[e].T.astype(np_dt)
            if len(t):
                blk = np.zeros((P, KTc, caps[j]), np_dt)
                blk[:, :, :len(t)] = x[t].T.reshape(
                    KTc, P, len(t)).transpose(1, 0, 2).astype(np_dt)
                xgT[xg_offs[j]:xg_offs[j + 1]] = blk.ravel()
                wv[j, :len(t)] = w
                rows = (core * S + j) * CAPMAX + np.arange(len(t))
                inv[t, cnt[t]] = rows
                cnt[t] += 1
        q, th = core % 4, core // 4
        i0 = q * ISS
        sguT = np.concatenate(
            [shared_gate_up[i0:i0 + ISS].T,
             shared_gate_up[IS + i0:IS + i0 + ISS].T], axis=1).astype(np_dt)
        sdT = shared_down[:, i0:i0 + ISS].T.astype(np_dt)
        in_maps.append({
            "xgT": xgT, "w1T": w1T, "w2T": w2T,
            "wts": wv, "xT": np.ascontiguousarray(xT_np[:, th * MH:(th + 1) * MH]),
            "sguT": np.ascontiguousarray(sguT),
            "sdT": np.ascontiguousarray(sdT),
        })
    assert (cnt == TOPK).all()
    return nc, in_maps, (S, CAPMAX, inv)


def _unshard(results, meta):
    S, CAPMAX, inv = meta
    ywc = np.concatenate(
        [results[c]["yw"].reshape(S * CAPMAX, H) for c in range(NCORES)])
    out = ywc[inv.ravel()].reshape(M, TOPK, H).sum(axis=1, dtype=np.float64)
    MH_ = M // 2
    for c in range(NCORES):
        th = c // 4
        out[th * MH_:(th + 1) * MH_] += results[c]["sh"].astype(np.float64)
    return out.astype(np.float32)


def kernel(x, gate_w, w1, w2, shared_gate_up, shared_down):
    nc, in_maps, meta = _prepare(x, gate_w, w1, w2,
                                 shared_gate_up, shared_down)
    res = bass_utils.run_bass_kernel_spmd(
        nc, in_maps, core_ids=list(range(NCORES)))
    return _unshard(res.results, meta)



# revision 2
# speedup vs baseline: 1.7808x; 1.7808x over previous
"""Trainium2 Bass kernel for BlockSparseMoE (DeepSeek-V2-Lite-like MoE layer).

Strategy (8 NeuronCores, SPMD):
  * Router (softmax + grouped top-k over [2048, 64] scores) is computed on
    host in numpy - it is ~0.03% of the FLOPs; selection matches the jax
    reference exactly on the fixed inputs.
  * Routed experts run in fp8 (e4m3) with DoubleRow matmuls: the shared
    expert's output dominates the total output magnitude by >10x, so fp8
    quantization error on the routed path is diluted ~12x and the end-to-end
    relative error stays ~7e-3 (gate 2e-2). Weights are pre-scaled by 16 on
    host so they sit in e4m3's normal range; the silu unscales by 1/16 and
    the final copy emits 8*y (host divides by 8 when applying the combine
    weights). fp8 halves HBM traffic vs bf16 (the binding constraint) and
    quadruples matmul throughput in DoubleRow mode.
  * Expert-parallel dispatch: each core owns 8 of the 64 experts; tokens are
    gathered per expert on host, rank-matched across cores so the SPMD
    program is shape-uniform (2.6% padding). Combine weights are applied on
    host during the unshard (free, and keeps fp8 outputs in range).
  * Shared experts stay bf16 (accuracy-critical): 2D-sharded
    (intermediate-quarter x token-half per core); partial outputs summed on
    host. Their matmuls are interleaved between routed slots as PE filler so
    the tensor engine never idles while routed weights stream in.
"""

import sys
import math

sys.path.insert(0, "/opt/trn_rl_repo")

import numpy as np
import ml_dtypes

import concourse.bass as bass  # noqa: F401  (registers AP machinery)
import concourse.mybir as mybir
import concourse.tile as tile
from concourse import bacc
from concourse import bass_utils

# Model dims (hardcoded per spec)
M = 2048
H = 1024
E = 64
N = 1024
N_GROUP = 8
TOPK_GROUP = 3
TOPK = 6
IS = 2048          # shared-expert intermediate size (n_shared=2 * N)
NCORES = 8
ISS = IS // 4      # per-core shared-expert intermediate slice (2D shard)
MH = M // 2        # per-core shared-expert token half
MAXC = 256         # max tokens per expert chunk (DoubleRow moving-dim limit)
SW = 16.0          # fp8 weight pre-scale (keeps w*16 in e4m3 normal range)

P = 128
KT = H // P        # 8 k-tiles for H contraction
KP = KT // 2       # 4 DoubleRow k-tile pairs
NT = N // P        # 8 n-tiles for N contraction
FT = ISS // P      # shared-expert f-tiles per gate/up half (4)

f8 = mybir.dt.float8e4
bf = mybir.dt.bfloat16
f32 = mybir.dt.float32
npf8 = ml_dtypes.float8_e4m3
npbf = ml_dtypes.bfloat16


# ---------------------------------------------------------------- routing ---
def _route(x, gate_w):
    """Numpy replica of the reference router. Returns topk ids/weights."""
    logits = x @ gate_w.T                          # [M, E] fp32 sgemm
    mx = logits.max(-1, keepdims=True)
    ex = np.exp(logits - mx)
    scores = ex / ex.sum(-1, keepdims=True)        # softmax, [M, E]
    m = scores.shape[0]
    gs = scores.reshape(m, N_GROUP, E // N_GROUP).max(-1)        # [M, G]
    gidx = np.argsort(-gs, axis=-1, kind="stable")[:, :TOPK_GROUP]
    gmask = np.zeros((m, N_GROUP), bool)
    gmask[np.arange(m)[:, None], gidx] = True
    smask = np.repeat(gmask, E // N_GROUP, axis=1)               # [M, E]
    tmp = np.where(smask, scores, 0.0)
    topk_ids = np.argsort(-tmp, axis=-1, kind="stable")[:, :TOPK]
    topk_w = np.take_along_axis(tmp, topk_ids, axis=-1)
    return topk_w.astype(np.float32), topk_ids


# ------------------------------------------------------------ bass program ---
_prog_cache = {}


def _build_program(S, caps, xo, yo):
    """One SPMD program: S fp8 expert slots (slot j holds caps[j] tokens)
    plus a bf16 shared-expert slice, with shared-expert pieces interleaved
    between slots so the PE stays busy while routed weights stream in."""
    key = (S, tuple(caps))
    if key in _prog_cache:
        return _prog_cache[key]

    DRmode = mybir.MatmulPerfMode.DoubleRow
    Silu = mybir.ActivationFunctionType.Silu
    Copy = mybir.ActivationFunctionType.Copy
    YW = yo[-1]

    nc = bacc.Bacc("TRN2", target_bir_lowering=False, debug=False,
                   num_devices=NCORES)

    xgT = nc.dram_tensor("xgT", [xo[-1]], f8, kind="ExternalInput")
    w1T = nc.dram_tensor("w1T", [S, H, 2 * N], f8, kind="ExternalInput")
    w2T = nc.dram_tensor("w2T", [S, N, H], f8, kind="ExternalInput")
    xT = nc.dram_tensor("xT", [H, MH], bf, kind="ExternalInput")
    sguT = nc.dram_tensor("sguT", [H, 2 * ISS], bf, kind="ExternalInput")
    sdT = nc.dram_tensor("sdT", [ISS, H], bf, kind="ExternalInput")
    yw = nc.dram_tensor("yw", [YW, H], f8, kind="ExternalOutput")
    sh = nc.dram_tensor("sh", [MH, H], bf, kind="ExternalOutput")

    with tile.TileContext(nc) as tc:
        with (
            tc.tile_pool(name="w1pool", bufs=3) as w1pool,
            tc.tile_pool(name="wpool", bufs=3) as wpool,
            tc.tile_pool(name="xpool", bufs=3) as xpool,
            tc.tile_pool(name="hpool", bufs=2) as hpool,
            tc.tile_pool(name="spool", bufs=1) as spool,
            tc.tile_pool(name="silu", bufs=3) as silu_pool,
            tc.tile_pool(name="opool", bufs=2) as opool,
            tc.tile_pool(name="ps1", bufs=4, space="PSUM") as ps1,
            tc.tile_pool(name="ps2", bufs=3, space="PSUM") as ps2,
        ):
            state = {}

            def routed_slot(s):
                cap = caps[s]
                ct_n = math.ceil(cap / P)
                xg_sb = xpool.tile([P, KT, cap], f8, tag="xg", name="xg_sb")
                nc.sync.dma_start(
                    xg_sb[:],
                    xgT.ap()[xo[s]:xo[s + 1]].rearrange(
                        "(p kt c) -> p kt c", p=P, kt=KT))
                w1_sb = w1pool.tile([P, KT, 2 * N], f8, tag="w1",
                                    name="w1_sb")
                w1r = w1T.ap()[s].rearrange("(kt p) f -> p kt f", p=P)
                for half in range(2):
                    hs = slice(half * (KT // 2), (half + 1) * (KT // 2))
                    nc.sync.dma_start(w1_sb[:, hs], w1r[:, hs])
                w2_sb = wpool.tile([P, NT, H], f8, tag="w2", name="w2_sb")
                nc.sync.dma_start(
                    w2_sb[:], w2T.ap()[s].rearrange("(nt p) o -> p nt o", p=P))

                # gate/up GEMM (fp8 DoubleRow) -> silu*mul -> hT fp8 in SBUF
                h_sb = hpool.tile([P, NT, MAXC], f8, tag="h", name="h_sb")
                for nt in range(NT):
                    pgu = ps1.tile([P, 512], f32, tag="ps1", name="pgu")
                    pg = pgu[:, :cap]
                    pu = pgu[:, 256:256 + cap]
                    for kp in range(KP):
                        nc.tensor.matmul(
                            pg, w1_sb[:, 2 * kp:2 * kp + 2, nt * P:(nt + 1) * P],
                            xg_sb[:, 2 * kp:2 * kp + 2],
                            start=(kp == 0), stop=(kp == KP - 1),
                            perf_mode=DRmode)
                    for kp in range(KP):
                        nc.tensor.matmul(
                            pu,
                            w1_sb[:, 2 * kp:2 * kp + 2,
                                  N + nt * P:N + (nt + 1) * P],
                            xg_sb[:, 2 * kp:2 * kp + 2],
                            start=(kp == 0), stop=(kp == KP - 1),
                            perf_mode=DRmode)
                    st_t = silu_pool.tile([P, MAXC], f32, tag="silu",
                                          name="st_t")
                    st = st_t[:, :cap]
                    nc.scalar.activation(st, pg, Silu, scale=1.0 / SW)
                    nc.vector.tensor_mul(h_sb[:, nt, :cap], st, pu)

                # down GEMM (fp8 DoubleRow) -> 8*y fp8 -> yw
                ow = opool.tile([P, 2, H], f8, tag="ow", name="ow")
                for ct in range(ct_n):
                    pt = min(P, cap - ct * P)
                    for ocp in range(2):
                        pod = ps2.tile([P, 512], f32, tag="ps2", name="pod")
                        for oc2 in range(2):
                            oc = ocp * 2 + oc2
                            po = pod[:pt, oc2 * 256:(oc2 + 1) * 256]
                            for kp in range(KP):
                                nc.tensor.matmul(
                                    po,
                                    h_sb[:, 2 * kp:2 * kp + 2,
                                         ct * P:ct * P + pt],
                                    w2_sb[:, 2 * kp:2 * kp + 2,
                                          oc * 256:(oc + 1) * 256],
                                    start=(kp == 0), stop=(kp == KP - 1),
                                    perf_mode=DRmode)
                        # psum holds 256*y; emit 8*y (ACT/DVE split by ct)
                        if ct == 0:
                            nc.scalar.activation(
                                ow[:pt, ct, ocp * 512:(ocp + 1) * 512],
                                pod[:pt], Copy, scale=1.0 / 32.0)
                        else:
                            nc.vector.tensor_scalar_mul(
                                ow[:pt, ct, ocp * 512:(ocp + 1) * 512],
                                pod[:pt], 1.0 / 32.0)
                    nc.scalar.dma_start(
                        yw.ap()[yo[s] + ct * P:yo[s] + ct * P + pt],
                        ow[:pt, ct])

            def g1_load(off, W=512):
                xT_t = xpool.tile([P, KT, 512], bf, tag="xTc", name="xT_t")
                xT_sb = xT_t[:, :, :W]
                xr = xT.ap().rearrange("(kt p) m -> p kt m", p=P)
                if off == 0:
                    sgu_sb = spool.tile([P, KT, 2 * ISS], bf, tag="sgu",
                                        name="sgu_sb")
                    sgur = sguT.ap().rearrange("(kt p) f -> p kt f", p=P)
                    # interleave per-kt so the first matmuls start early
                    for kt in range(KT):
                        nc.sync.dma_start(xT_sb[:, kt], xr[:, kt, off:off + W])
                        nc.sync.dma_start(sgu_sb[:, kt], sgur[:, kt])
                    state["sgu"] = sgu_sb
                else:
                    nc.sync.dma_start(xT_sb[:], xr[:, :, off:off + W])
                sh_h = hpool.tile([P, FT, 512], bf, tag="shh", name="sh_h")
                state[("x", off)] = xT_sb
                state[("h", off)] = sh_h

            def g1_piece(off, i, W=512):
                sgu_sb = state["sgu"]
                xT_sb = state[("x", off)]
                sh_h = state[("h", off)]
                pg_t = ps1.tile([P, 512], f32, tag="ps1", name="pg_t")
                pg = pg_t[:, :W]
                pu_t = ps1.tile([P, 512], f32, tag="ps1", name="pu_t")
                pu = pu_t[:, :W]
                for kt in range(KT):
                    nc.tensor.matmul(
                        pg, sgu_sb[:, kt, i * P:(i + 1) * P], xT_sb[:, kt],
                        start=(kt == 0), stop=(kt == KT - 1))
                for kt in range(KT):
                    nc.tensor.matmul(
                        pu, sgu_sb[:, kt, ISS + i * P:ISS + (i + 1) * P],
                        xT_sb[:, kt],
                        start=(kt == 0), stop=(kt == KT - 1))
                st_t = silu_pool.tile([P, 512], f32, tag="silus",
                                      name="st_t2")
                st = st_t[:, :W]
                nc.scalar.activation(st, pg, Silu)
                nc.vector.tensor_mul(sh_h[:, i, :W], st, pu)

            def g2_load():
                sd_sb = spool.tile([P, FT, H], bf, tag="sd", name="sd_sb")
                nc.sync.dma_start(
                    sd_sb[:], sdT.ap().rearrange("(nt p) o -> p nt o", p=P))
                state["sd"] = sd_sb

            def g2_piece(off, ctc):
                sd_sb = state["sd"]
                sh_h = state[("h", off)]
                os_t = opool.tile([P, H], bf, tag="osh", name="os_t")
                for oc in range(2):
                    pod = ps2.tile([P, 512], f32, tag="ps2", name="pod2")
                    for nt2 in range(FT):
                        nc.tensor.matmul(
                            pod[:], sh_h[:, nt2, ctc * P:(ctc + 1) * P],
                            sd_sb[:, nt2, oc * 512:(oc + 1) * 512],
                            start=(nt2 == 0), stop=(nt2 == FT - 1))
                    nc.vector.tensor_copy(os_t[:, oc * 512:(oc + 1) * 512],
                                          pod[:])
                nc.scalar.dma_start(
                    sh.ap()[off + ctc * P:off + (ctc + 1) * P], os_t[:])

            # schedule: open with a shared gate/up block (small input
            # footprint covers the routed weight-stream ramp), then routed
            # slots with shared pieces as PE filler between them; end on the
            # smallest routed slot to keep the drain tail short.
            def do(a):
                if a[0] == "slot":
                    routed_slot(a[1])
                elif a[0] == "g1l":
                    g1_load(a[1])
                elif a[0] == "g1":
                    g1_piece(a[1], a[2])
                elif a[0] == "g2l":
                    g2_load()
                else:
                    g2_piece(a[1], a[2])

            fillers = [
                [("g1l", 512), ("g1", 512, 0), ("g1", 512, 1)],
                [("g1", 512, 2), ("g1", 512, 3)],
                [("g2l",), ("g2", 0, 0), ("g2", 0, 1)],
                [("g2", 0, 2), ("g2", 0, 3)],
                [("g2", 512, 0), ("g2", 512, 1)],
                [("g2", 512, 2)],
                [("g2", 512, 3)],
            ]
            do(("g1l", 0))
            for i in range(FT):
                do(("g1", 0, i))
            for s in range(S):
                do(("slot", s))
                if s < S - 1:
                    for a in (fillers.pop(0) if fillers else []):
                        do(a)
            for grp in fillers:  # S too small: drain remaining shared work
                for a in grp:
                    do(a)

    nc.compile()
    _prog_cache[key] = nc
    return nc


# ------------------------------------------------------------------ kernel ---
def _prepare(x, gate_w, w1, w2, shared_gate_up, shared_down):
    x = np.ascontiguousarray(np.asarray(x, np.float32))
    gate_w = np.asarray(gate_w, np.float32)
    w1 = np.asarray(w1, np.float32)
    w2 = np.asarray(w2, np.float32)
    shared_gate_up = np.asarray(shared_gate_up, np.float32)
    shared_down = np.asarray(shared_down, np.float32)

    # ---- host router + dispatch build
    topk_w, topk_ids = _route(x, gate_w)
    order = np.argsort(topk_ids, axis=None, kind="stable")  # stable (t, k)
    flat_ids = topk_ids.ravel()[order]
    flat_tok = (np.arange(M * TOPK) // TOPK)[order]
    flat_w = topk_w.ravel()[order]
    starts = np.searchsorted(flat_ids, np.arange(E + 1))
    chunks = []  # (ntok, expert, tokens, weights)
    for e in range(E):
        t = flat_tok[starts[e]:starts[e + 1]]
        w = flat_w[starts[e]:starts[e + 1]]
        for i in range(0, max(len(t), 1), MAXC):
            chunks.append((len(t[i:i + MAXC]), e, t[i:i + MAXC],
                           w[i:i + MAXC]))

    # rank-match chunks across cores: sort by size, chunk ranked r goes to
    # core r%8, slot r//8 -> slot j has capacity max(sizes of ranks 8j..8j+7)
    chunks.sort(key=lambda c: -c[0])
    S = math.ceil(len(chunks) / NCORES)
    while len(chunks) < S * NCORES:
        chunks.append((0, 0, np.zeros(0, np.int64), np.zeros(0, np.float32)))
    caps = [max(16, chunks[j * NCORES][0]) for j in range(S)]
    xo = [0]
    yo = [0]
    for c in caps:
        xo.append(xo[-1] + P * KT * c)
        yo.append(yo[-1] + c)
    YW = yo[-1]

    nc = _build_program(S, caps, xo, yo)

    # ---- per-core input maps
    xT_np = np.ascontiguousarray(x.T).astype(npbf)
    in_maps = []
    inv = np.zeros((M, TOPK), np.int64)
    winv = np.zeros((M, TOPK), np.float32)
    cnt = np.zeros(M, np.int32)
    for core in range(NCORES):
        xgT = np.zeros(xo[-1], npf8)
        w1T = np.zeros((S, H, 2 * N), npf8)
        w2T = np.zeros((S, N, H), npf8)
        for j in range(S):
            _, e, t, w = chunks[j * NCORES + core]
            w1T[j] = (w1[e].T * SW).astype(npf8)
            w2T[j] = (w2[e].T * SW).astype(npf8)
            if len(t):
                blk = np.zeros((P, KT, caps[j]), npf8)
                blk[:, :, :len(t)] = x[t].T.reshape(
                    KT, P, len(t)).transpose(1, 0, 2).astype(npf8)
                xgT[xo[j]:xo[j + 1]] = blk.ravel()
                rows = core * YW + yo[j] + np.arange(len(t))
                inv[t, cnt[t]] = rows
                winv[t, cnt[t]] = w / 8.0  # yw holds 8*y
                cnt[t] += 1
        q, th = core % 4, core // 4
        i0 = q * ISS
        sguT = np.concatenate(
            [shared_gate_up[i0:i0 + ISS].T,
             shared_gate_up[IS + i0:IS + i0 + ISS].T], axis=1).astype(npbf)
        sdT = shared_down[:, i0:i0 + ISS].T.astype(npbf)
        in_maps.append({
            "xgT": xgT, "w1T": w1T, "w2T": w2T,
            "xT": np.ascontiguousarray(xT_np[:, th * MH:(th + 1) * MH]),
            "sguT": np.ascontiguousarray(sguT),
            "sdT": np.ascontiguousarray(sdT),
        })
    assert (cnt == TOPK).all()
    return nc, in_maps, (YW, inv, winv)


def _unshard(results, meta):
    YW, inv, winv = meta
    ywc = np.concatenate(
        [results[c]["yw"].astype(np.float32) for c in range(NCORES)])
    gathered = ywc[inv.ravel()].reshape(M, TOPK, H)
    out = (gathered * winv.reshape(M, TOPK, 1)).sum(axis=1, dtype=np.float64)
    for c in range(NCORES):
        th = c // 4
        out[th * MH:(th + 1) * MH] += results[c]["sh"].astype(np.float64)
    return out.astype(np.float32)


def kernel(x, gate_w, w1, w2, shared_gate_up, shared_down):
    nc, in_maps, meta = _prepare(x, gate_w, w1, w2,
                                 shared_gate_up, shared_down)
    res = bass_utils.run_bass_kernel_spmd(
        nc, in_maps, core_ids=list(range(NCORES)))
    return _unshard(res.results, meta)
